# revision 16
# baseline (speedup 1.0000x reference)
"""MultiHeadCrossAttention TRN2 kernel v2 (8 NeuronCores, SPMD).

Sharding: core c -> (batch b = c // 2, head-half hh = c % 2).
Head-half hh owns heads [hh, hh+2, ..., hh+14].

v2 redesign vs v1:
- q-major attention: att psum tiles are [128 q, 65] (K=128 contraction,
  N=65) -- half the PE cycles of the old [65, 2048] layout, and
  normalization becomes a per-partition tensor_scalar multiply.
- probs columns parity-reordered (col = 512 e + u for q = 1024 half + 2u + e)
  so each contiguous 128-wide probs slice is a single parity class; the
  post-att PE transpose (output partition base 64 e) lands directly in the
  output-linear contraction layout attn_t[(64 e + d), jj, r].
- keep mask resident in SBUF, reordered to match -- no per-head keep DMA.
- q processed in two halves; attention split into pass1 (kt 0-7) and pass2
  (kt 8-15) with one PSUM accumulation group per 2 KB zero-region.
- ACT-paced position loop: scores emitted one position ahead; all other PE
  work (att passes, projections, output linear) flows through a fine-grained
  work queue pumped between positions so neither PE nor ACT head-of-line
  blocks.
- engine split: PE matmuls only; ACT exp; DVE mask-mult + recip + normalize;
  Pool (gpsimd) bias adds + PSUM->SBUF copies.
"""

from collections import deque

import numpy as np

B, S, D, H, HD = 4, 2048, 1024, 16, 64
NCORES = 8
NKT = D // 128  # 8 K-tiles over the enc feature dim

_CACHE = {}
FG_BUDGET = 3   # att-pass generator yields pumped per kt position
BG_BUDGET = 2   # projection/linear generator yields pumped per kt position
POOL_KT = 3     # mask-mults for kt < POOL_KT run on Pool (gpsimd)
WSCALE = 16.0   # fp8 projection-weight pre-scale (undone in the bias add)


def _heads_for(hh):
    return list(range(hh, H, 2))


# global parity permutation: new index s' = e*1024 + u  <->  old s = 2u + e
_PERM = np.arange(S).reshape(1024, 2).T.reshape(S)  # old s for each new s'


def _build_nc(nslots=8, nreps=1, probs_bufs=13, kt_slice=True, debug=False,
              debug_slot=0):
    import concourse.bass as bass
    import concourse.tile as tile
    from concourse import bacc, mybir

    f32 = mybir.dt.float32
    bf16 = mybir.dt.bfloat16
    f8 = mybir.dt.float8e4
    DR = mybir.MatmulPerfMode.DoubleRow
    MUL, ADD = mybir.AluOpType.mult, mybir.AluOpType.add
    ts, ds = bass.ts, bass.ds
    Exp = mybir.ActivationFunctionType.Exp

    nc = bacc.Bacc("TRN2", target_bir_lowering=False, debug=False,
                   num_devices=NCORES)

    # projections run fp8 DoubleRow: xt = 8 enc chunk planes packed as 4
    # contraction pairs; the extra (dec/identity, pair-2-only) chunk stays
    # bf16 via xt8/wqk8.  fp8 weights are pre-scaled x16 (else ~N(0,.02)
    # lands subnormal in e4m3); the bias add folds in the 1/16.
    xt_d = nc.dram_tensor("xt", [128, 8, S], f8, kind="ExternalInput")
    xt8_d = nc.dram_tensor("xt8", [128, S], bf16, kind="ExternalInput")
    wqk_d = nc.dram_tensor("wqk", [2, 128, 3, 4, 2, 128], f8,
                           kind="ExternalInput")
    wqk8_d = nc.dram_tensor("wqk8", [128, 2, 128], bf16,
                            kind="ExternalInput")
    qkb_d = nc.dram_tensor("qkb", [2, 128, 3], f32, kind="ExternalInput")
    wv_d = nc.dram_tensor("wv", [128, 3, 4, 2, 128], f8,
                          kind="ExternalInput")
    vbp_d = nc.dram_tensor("vbp", [128, 3], f32, kind="ExternalInput")
    qkdecT_d = nc.dram_tensor("qkdect", [2, 128, S], bf16,
                              kind="ExternalInput")
    vdec_d = nc.dram_tensor("vdec", [128, 16, 3, 65], bf16,
                            kind="ExternalInput")
    keep_d = nc.dram_tensor("keep", [128, 16, S], bf16, kind="ExternalInput")
    lin_d = nc.dram_tensor("lin", [128, 8, D], bf16, kind="ExternalInput")
    linb_d = nc.dram_tensor("linb", [1, D], bf16, kind="ExternalInput")
    out_d = nc.dram_tensor("out", [8, 128, D], bf16, kind="ExternalOutput")
    if debug:
        dqkt_d = nc.dram_tensor("dqkt", [128, S], bf16, kind="ExternalOutput")
        dprobs_d = nc.dram_tensor("dprobs", [16, 128, 1024], bf16,
                                  kind="ExternalOutput")
        dattn_d = nc.dram_tensor("dattn", [128, 8, 128], bf16,
                                 kind="ExternalOutput")
        dvaug_d = nc.dram_tensor("dvaug", [128, 16, 65], bf16,
                                 kind="ExternalOutput")
        dan_d = nc.dram_tensor("dan", [8, 128, 64], bf16,
                               kind="ExternalOutput")
        dpa_d = nc.dram_tensor("dpa", [8, 128, 65], f32,
                               kind="ExternalOutput")

    with tile.TileContext(nc) as tc:
        with (
            tc.tile_pool(name="consts", bufs=1) as consts,
            tc.tile_pool(name="qkt", bufs=4) as qktp,
            tc.tile_pool(name="vt2", bufs=1) as vt2p,  # [128, 1024] halves
            tc.tile_pool(name="vaug", bufs=3) as vaugp,
            tc.tile_pool(name="probs", bufs=probs_bufs) as probsp,
            tc.tile_pool(name="anorm", bufs=2) as anormp,
            tc.tile_pool(name="attn", bufs=2) as attnp,
            tc.tile_pool(name="rst", bufs=1) as rstp,
            tc.tile_pool(name="sums", bufs=2) as sumsp,
            tc.tile_pool(name="ps_sc", bufs=2, space="PSUM") as ps_sc,
            tc.tile_pool(name="ps_pa", bufs=1, space="PSUM") as ps_pa,
            tc.tile_pool(name="ps_pj", bufs=1, space="PSUM") as ps_pj,
            tc.tile_pool(name="ps_pg", bufs=1, space="PSUM") as ps_pg,
        ):
            # ---- resident constants.  DMA issue order follows first use:
            # slots 6,7 (dec-sourced) run first, so qkdecT/vdec/keep lead;
            # xt/wqk (needed by the first projections, pulled from slot 6
            # half 1 on) follow; lin/linb are only needed at slot ends. ----
            # dec-sourced heads: q-plane tile and k-plane tile, slot 6 on
            # partitions 0:64 and slot 7 on 64:128 of each
            qdec_sb = consts.tile([128, S], bf16)
            nc.sync.dma_start(out=qdec_sb[:], in_=qkdecT_d[0, :, :])
            kdec_sb = consts.tile([128, S], bf16)
            nc.sync.dma_start(out=kdec_sb[:], in_=qkdecT_d[1, :, :])
            vdec_sb = consts.tile([128, 16, 3, 65], bf16)
            nc.sync.dma_start(out=vdec_sb[:], in_=vdec_d[:, :, :, :])
            keep_sb = consts.tile([128, 16, S], bf16)
            for i in range(4):
                nc.sync.dma_start(out=keep_sb[:, 4 * i:4 * i + 4, :],
                                  in_=keep_d[:, 4 * i:4 * i + 4, :])
            xt_sb = consts.tile([128, 8, S], f8)
            nc.sync.dma_start(out=xt_sb[:, 0:4, :], in_=xt_d[:, 0:4, :])
            nc.sync.dma_start(out=xt_sb[:, 4:8, :], in_=xt_d[:, 4:8, :])
            xt8_sb = consts.tile([128, S], bf16)
            nc.sync.dma_start(out=xt8_sb[:], in_=xt8_d[:, :])
            wqkq_sb = consts.tile([128, 3, 4, 2, 128], f8)
            nc.sync.dma_start(out=wqkq_sb[:], in_=wqk_d[0, :, :, :, :, :])
            wqkk_sb = consts.tile([128, 3, 4, 2, 128], f8)
            nc.sync.dma_start(out=wqkk_sb[:], in_=wqk_d[1, :, :, :, :, :])
            wqk8_sb = consts.tile([128, 2, 128], bf16)
            nc.sync.dma_start(out=wqk8_sb[:], in_=wqk8_d[:, :, :])
            qkbq_sb = consts.tile([128, 3], f32)
            nc.sync.dma_start(out=qkbq_sb[:], in_=qkb_d[0, :, :])
            qkbk_sb = consts.tile([128, 3], f32)
            nc.sync.dma_start(out=qkbk_sb[:], in_=qkb_d[1, :, :])
            wv_sb = consts.tile([128, 3, 4, 2, 128], f8)
            nc.sync.dma_start(out=wv_sb[:], in_=wv_d[:, :, :, :, :])
            vbp_sb = consts.tile([128, 3], f32)
            nc.sync.dma_start(out=vbp_sb[:], in_=vbp_d[:, :])
            lin_sb = consts.tile([128, 8, D], bf16)
            nc.sync.dma_start(out=lin_sb[:], in_=lin_d[:, :, :])
            linb_sb = consts.tile([1, D], bf16)
            nc.sync.dma_start(out=linb_sb[:], in_=linb_d[:, :])
            ones_sb = consts.tile([1, 128], bf16)
            nc.vector.memset(ones_sb[:], 1.0)
            ident_sb = consts.tile([128, 128], bf16)
            from concourse.masks import make_identity
            make_identity(nc, ident_sb[:])

            QK = {}    # slot % 2 -> (qkT, kT-or-None)
            VAUG = {}  # slot -> vaug tile
            GENS = {}  # ("qk", slot) / ("v", pair) -> generator in `work`
            debug_ref = ((debug_slot, 0), dan_d, dpa_d) if debug else None

            def gen_qkproj(pair):
                """paired qk projection for slots (2 pair, 2 pair + 1):
                qtile [128, S] = (qA on 0:64, qB on 64:128), ktile likewise.
                fp8 DoubleRow over 4 chunk-pairs; pair 2 adds a bf16 extra
                chunk from xt8 (slot-5 identity path on head-half 1; zero
                weights elsewhere).  The 1/WSCALE un-scale rides the bias
                add."""
                qtile = qktp.tile([128, S], bf16, tag="qkt")
                ktile = qktp.tile([128, S], bf16, tag="qkt")
                for qk in range(2):
                    dst = qtile if qk == 0 else ktile
                    wsb = wqkq_sb if qk == 0 else wqkk_sb
                    bsb = qkbq_sb if qk == 0 else qkbk_sb
                    for c in range(4):
                        pq = ps_pj.tile([128, 512], f32, tag="ps_pj")
                        for cp in range(4):
                            nc.tensor.matmul(
                                pq[:],
                                lhsT=wsb[:, pair, cp, :, :],
                                rhs=xt_sb[:, ds(2 * cp, 2), ds(c * 512, 512)],
                                perf_mode=DR,
                                start=(cp == 0), stop=(cp == 3 and pair != 2),
                                skip_group_check=True,
                            )
                            if cp % 2 == 1:
                                yield
                        if pair == 2:
                            nc.tensor.matmul(
                                pq[:],
                                lhsT=wqk8_sb[:, qk, :],
                                rhs=xt8_sb[:, ds(c * 512, 512)],
                                start=False, stop=True,
                                skip_group_check=True,
                            )
                        nc.vector.tensor_scalar(
                            dst[:, ds(c * 512, 512)], pq[:],
                            1.0 / WSCALE, bsb[:, pair:pair + 1], MUL, ADD,
                        )
                        yield
                QK[pair] = (qtile, ktile)

            def gen_vproj(pair):
                """pair-packed v projection -> vaug tiles for slots
                (2 pair, 2 pair + 1); fp8 DoubleRow, fine-grained yields."""
                vaugA = vaugp.tile([128, 16, 65], bf16, tag="vaug")
                vaugB = vaugp.tile([128, 16, 65], bf16, tag="vaug")
                nc.gpsimd.memset(vaugA[:, :, 64:65], 1.0)
                nc.gpsimd.memset(vaugB[:, :, 64:65], 1.0)
                for ch in range(2):
                    vT2 = vt2p.tile([128, 1024], bf16, tag="vt2")
                    for cc in range(2):
                        c = 2 * ch + cc
                        pv = ps_pj.tile([128, 512], f32, tag="ps_pj")
                        for cp in range(4):
                            nc.tensor.matmul(
                                pv[:],
                                lhsT=wv_sb[:, pair, cp, :, :],
                                rhs=xt_sb[:, ds(2 * cp, 2), ds(c * 512, 512)],
                                perf_mode=DR,
                                start=(cp == 0), stop=(cp == 3),
                                skip_group_check=True,
                            )
                            if cp % 2 == 1:
                                yield
                        nc.vector.tensor_scalar(
                            vT2[:, ds(cc * 512, 512)], pv[:],
                            1.0 / WSCALE, vbp_sb[:, pair:pair + 1], MUL, ADD,
                        )
                    for tq in range(2 * ch, 2 * ch + 2):
                        pt = ps_pg.tile([128, 4, 128], bf16, tag="ps_pg")
                        for t in range(4):
                            nc.tensor.transpose(
                                pt[:, t, :],
                                vT2[:, ds((4 * tq + t) * 128 - ch * 1024,
                                          128)],
                                ident_sb[:, :])
                        nc.vector.tensor_copy(vaugA[:, ds(4 * tq, 4), 0:64],
                                              pt[:, :, 0:64])
                        nc.vector.tensor_copy(vaugB[:, ds(4 * tq, 4), 0:64],
                                              pt[:, :, 64:128])
                        yield
                VAUG[2 * pair] = vaugA
                VAUG[2 * pair + 1] = vaugB

            def vaug_for(slot):
                if slot < 5:
                    return VAUG[slot]
                return vdec_sb[:, :, slot - 5, :]

            def qk_aps(slot):
                if slot >= 6:
                    qtile, ktile = qdec_sb, kdec_sb
                    sub = slot - 6
                else:
                    qtile, ktile = QK[slot // 2]
                    sub = slot % 2
                lo = 64 * sub
                return qtile[lo:lo + 64, :], ktile[lo:lo + 64, :]

            def emit_scores(slot, half, kt, psc):
                qT, kT = qk_aps(slot)
                for cc in range(2):
                    nc.tensor.matmul(
                        psc[:, ds(cc * 512, 512)],
                        lhsT=kT[:, ts(kt, 128)],
                        rhs=qT[:, ds(half * 1024 + cc * 512, 512)],
                        start=True, stop=True, skip_group_check=True,
                    )

            def gen_att_pass1(slot, probs_tiles, pa_all, kts):
                # PSUM zero-regions are 2048 B: pa_all[:, 0:4, :] is one
                # region, pa_all[:, 4:8, :] the other.  start marks the WHOLE
                # region zero-on-first-write, so only the first group per
                # region (u = 0, 4) starts; only the last (u = 3, 7) stops.
                vaug = vaug_for(slot)
                for u in range(8):
                    pa = pa_all[:, u, :]
                    for kt in kts:
                        nc.tensor.matmul(
                            pa[:, 0:65],
                            lhsT=probs_tiles[kt][:, ts(u, 128)],
                            rhs=vaug[:, kt, :],
                            start=(kt == 0 and u % 4 == 0), stop=False,
                            skip_group_check=True,
                        )
                    yield

            def gen_att_pass2(slot, half, probs_tiles, pa_all, attn_t):
                # with the global parity reorder, e == half: partitions
                # 64*half of attn_t, r covered 0:128 across the 8 u-tiles
                vaug = vaug_for(slot)
                for tq in range(2):
                    pt = ps_pg.tile([128, 4, 128], bf16, tag="ps_pg")
                    for t in range(4):
                        u = 4 * tq + t
                        pa = pa_all[:, u, :]
                        for kt in range(12, 16):
                            nc.tensor.matmul(
                                pa[:, 0:65],
                                lhsT=probs_tiles[kt][:, ts(u, 128)],
                                rhs=vaug[:, kt, :],
                                start=False, stop=(kt == 15 and t == 3),
                                skip_group_check=True,
                            )
                        rc = sumsp.tile([128, 1], f32, tag="sums")
                        nc.vector.reciprocal(rc[:], pa[:, 64:65])
                        an = anormp.tile([128, 64], bf16, tag="anorm")
                        nc.vector.tensor_scalar_mul(an[:], pa[:, 0:64], rc[:])
                        nc.tensor.transpose(pt[ds(64 * half, 64), t, :],
                                            an[:], ident_sb[:, :])
                        if debug_ref and debug_ref[0] == (slot, half):
                            dstg = rstp.tile([128, 65], f32, tag="rst")
                            nc.vector.tensor_copy(dstg[:], pa[:, 0:65])
                            nc.sync.dma_start(out=debug_ref[2][u, :, :],
                                              in_=dstg[:])
                            nc.sync.dma_start(out=debug_ref[1][u, :, :],
                                              in_=an[:])
                        yield
                    # combined copy: src[p, t, (r j)] ->
                    # attn_t[64*half + p, j, 64*tq + 16 t + r]
                    csrc = pt[ds(64 * half, 64), :, :].rearrange(
                        "p t (r j) -> p j t r", j=8)
                    cdst = attn_t[ds(64 * half, 64), :,
                                  ds(64 * tq, 64)].rearrange(
                        "p j (t r) -> p j t r", r=16)
                    nc.vector.tensor_copy(cdst, csrc)
                    yield

            def gen_linear(slot, attn_t):
                if debug_ref and slot == debug_ref[0][0]:
                    nc.sync.dma_start(out=dattn_d[:], in_=attn_t[:])
                for n in range(2):
                    pr = ps_pg.tile([128, 512], f32, tag="ps_pg")
                    nc.tensor.matmul(
                        pr[:],
                        lhsT=ones_sb[0:1, :],
                        rhs=linb_sb[0:1, ds(n * 512, 512)],
                        start=True, stop=False, skip_group_check=True,
                    )
                    for jj in range(8):
                        nc.tensor.matmul(
                            pr[:],
                            lhsT=attn_t[:, jj, :],
                            rhs=lin_sb[:, jj, ds(n * 512, 512)],
                            start=False, stop=(jj == 7),
                            skip_group_check=True,
                        )
                        if jj % 3 == 2:
                            yield
                    rst = rstp.tile([128, 512], bf16, tag="rst")
                    nc.vector.tensor_copy(rst[:], pr[:])
                    nc.sync.dma_start(out=out_d[slot, :, ds(n * 512, 512)],
                                      in_=rst[:])
                    yield

            # ---------------- ACT-paced position scheduler -----------------
            # fg: att passes (latency-sensitive, consume probs).  bg:
            # projections + linear (slack work, pumped continuously so slot
            # boundaries never force-drain a cold projection).
            fg = deque()
            bg = deque()

            def pump_q(q, n):
                done = 0
                while done < n and q:
                    if next(q[0], "DONE") == "DONE":
                        q.popleft()
                    else:
                        done += 1

            def force_drain(key):
                # Exhaust the producer generator for `key` before a consumer
                # binds to its outputs (QK/VAUG are late-set dicts; emitting a
                # reader before the producer finishes would silently bind to
                # a stale tile).
                g = GENS.pop(key, None)
                if g is None:
                    return
                while g in bg:
                    pump_q(bg, 1)
                while g in fg:
                    pump_q(fg, 1)

            def drain_gen(g):
                for _ in g:
                    pass

            ORDER = [6, 7, 0, 1, 2, 3, 4, 5][:nslots] if nslots == 8 \
                else list(range(nslots))
            VPOS = {7: 0, 3: 1, 5: 2}  # position -> v pair built there

            QKPOS = {1: 0, 2: 1, 4: 2}  # position -> qk pair built there

            def enqueue_bg(si, rep):
                qp = QKPOS.get(si)
                if qp is not None and 2 * qp < min(nslots, 6):
                    g = gen_qkproj(qp)
                    GENS["qk", qp] = g
                    bg.append(g)
                pair = VPOS.get(si)
                if pair is not None and 2 * pair < min(nslots, 5):
                    g = gen_vproj(pair)
                    GENS["v", pair] = g
                    bg.append(g)

            pending_psc = None
            for rep in range(nreps):
                if rep == 0:
                    pending_psc = ps_sc.tile([128, 1024], f32, tag="ps_sc")
                    emit_scores(ORDER[0], 0, 0, pending_psc)
                for si, slot in enumerate(ORDER):
                    enqueue_bg(si, rep)
                    if rep == 0 and si == 1:
                        # steady-state builds v pair 0 at si=7 of the prior
                        # rep; bootstrap it here for the first rep
                        g = gen_vproj(0)
                        GENS["v", 0] = g
                        bg.append(g)

                    attn_t = attnp.tile([128, 8, 128], bf16, tag="attn")
                    for half in range(2):
                        probs_tiles = []
                        pa_all = ps_pa.tile([128, 8, 128], f32, tag="ps_pa")
                        for kt in range(16):
                            psc = pending_psc
                            pk = probsp.tile([128, 1024], bf16, tag="probs")
                            probs_tiles.append(pk)
                            nc.scalar.activation(
                                out=pk[:], in_=psc[:], func=Exp, scale=0.125,
                            )
                            # early-kt mask mults go to the (otherwise idle)
                            # Pool engine: their consumers (pass1a at kt==5)
                            # leave >2 us of slack for Pool's slower rate.
                            meng = nc.gpsimd if kt < POOL_KT else nc.vector
                            meng.tensor_mul(
                                pk[:], pk[:],
                                keep_sb[:, kt, ds(half * 1024, 1024)],
                            )
                            if debug and rep == 0 and slot == debug_slot \
                                    and half == 0:
                                nc.sync.dma_start(out=dprobs_d[kt, :, :],
                                                  in_=pk[:])
                            # software-pipelined next scores tile
                            if kt < 15:
                                nxt = (slot, half, kt + 1)
                            elif half == 0:
                                nxt = (slot, 1, 0)
                            elif si + 1 < len(ORDER):
                                nxt = (ORDER[si + 1], 0, 0)
                            elif rep + 1 < nreps:
                                nxt = (ORDER[0], 0, 0)
                            else:
                                nxt = None
                            if nxt is not None:
                                pending_psc = ps_sc.tile([128, 1024], f32,
                                                         tag="ps_sc")
                                emit_scores(*nxt, pending_psc)
                            if kt == 4 and slot < 5:
                                force_drain(("v", slot // 2))
                            if kt == 5:
                                fg.append(gen_att_pass1(
                                    slot, probs_tiles, pa_all, range(0, 4)))
                            elif kt == 9:
                                fg.append(gen_att_pass1(
                                    slot, probs_tiles, pa_all, range(4, 8)))
                            elif kt == 13:
                                fg.append(gen_att_pass1(
                                    slot, probs_tiles, pa_all, range(8, 12)))
                            elif kt == 10 and half == 1 and si + 1 < len(ORDER):
                                ns = ORDER[si + 1]
                                if ns < 6:
                                    force_drain(("qk", ns // 2))
                            elif kt == 15:
                                # linear must follow pass2 in the same FIFO:
                                # they share the single-buffered ps_pg pool,
                                # so interleaving them deadlocks PE<->DVE.
                                fg.append(gen_att_pass2(
                                    slot, half, probs_tiles, pa_all, attn_t))
                                if half == 1:
                                    fg.append(gen_linear(slot, attn_t))
                            pump_q(fg, FG_BUDGET)
                            pump_q(bg, BG_BUDGET)
                    if debug and rep == 0 and slot == debug_slot:
                        qkT0, kT0, _ = qk_aps(slot)
                        nc.sync.dma_start(out=dqkt_d[0:64, :], in_=qkT0)
                        nc.sync.dma_start(out=dqkt_d[64:128, :], in_=kT0)
                        nc.sync.dma_start(out=dvaug_d[:], in_=vaug_for(slot))
            while fg or bg:
                pump_q(fg, 1)
                pump_q(bg, 1)

    nc.compile()
    return nc


def _prep_core_inputs(b, hh, dec_input, enc_input, keep_r, W_qk_w, W_qk_b,
                      lin_in, lin_b16):
    import ml_dtypes
    bf16 = ml_dtypes.bfloat16
    heads = _heads_for(hh)
    enc_b = enc_input[b]
    dec_b = dec_input[b]
    encT = np.ascontiguousarray(enc_b.T[:, _PERM])  # [1024, 2048], s reordered

    xt = np.empty((9, 128, S), np.float32)
    xt[:NKT] = encT.reshape(NKT, 128, S)
    if hh == 0:
        xt[8] = encT[896:1024]
    else:
        # head 11 q,k dec cols 64:192 transposed
        xt[8] = np.ascontiguousarray(dec_b[:, 64:192].T[:, _PERM])
    xt8 = np.ascontiguousarray(xt[8])  # bf16 extra chunk [128, S]
    xt = np.ascontiguousarray(xt[:8].transpose(1, 0, 2))  # [128, 8, S] fp8

    # [2(q/k), 128, S]: plane 0 = q (slot 6 on partitions 0:64, slot 7 on
    # 64:128), plane 1 = k likewise
    qkdecT = np.empty((2, 128, S), np.float32)
    for i, slot in enumerate((6, 7)):
        h = heads[slot]
        mc = h * 192 - 2 * D  # dec col offset of this head's q
        qkdecT[0, 64 * i:64 * i + 64] = dec_b[:, mc:mc + 64].T[:, _PERM]
        qkdecT[1, 64 * i:64 * i + 64] = dec_b[:, mc + 64:mc + 128].T[:, _PERM]

    vdec = np.empty((128, 16, 3, 65), np.float32)
    vdec[:, :, :, 64] = 1.0
    for blk, slot in enumerate((5, 6, 7)):
        h = heads[slot]
        mcv = h * 192 + 128 - 2 * D
        vcols = dec_b[_PERM, mcv:mcv + 64]  # [2048, 64], k reordered
        vdec[:, :, blk, 0:64] = vcols.reshape(16, 128, 64).transpose(1, 0, 2)

    # paired qk weights: wqk[qk, :, pair, chunk, 0:64] = slot 2*pair,
    # [.., 64:128] = slot 2*pair+1.  Chunk NKT is the extra xt[8] chunk
    # (identity for the hh=1 slot-5 path, zero otherwise).  All weights are
    # pre-scaled x WSCALE for fp8 range; the kernel divides back.
    wqk = np.zeros((2, 128, 3, NKT + 1, 128), np.float32)
    qkb = np.zeros((2, 128, 3), np.float32)
    for slot in range(6):
        h = heads[slot]
        pair, sub = divmod(slot, 2)
        for qk in range(2):
            off = h * 192 + 64 * qk
            if hh == 1 and slot == 5:
                # q/k from dec cols (xt[8] holds dec[:, 64:192].T):
                # q = rows 0:64 of xt[8], k = rows 64:128
                wqk[qk, 64 * qk:64 * qk + 64, pair, NKT,
                    64 * sub:64 * sub + 64] = np.eye(64, dtype=np.float32)
            else:
                for p in range(NKT):
                    wqk[qk, :, pair, p, 64 * sub:64 * sub + 64] = \
                        W_qk_w[off:off + 64, p * 128:(p + 1) * 128].T
                qkb[qk, 64 * sub:64 * sub + 64, pair] = \
                    W_qk_b[off:off + 64]
    wqk *= WSCALE
    wqk_f8 = np.ascontiguousarray(
        wqk[:, :, :, 0:8, :].reshape(2, 128, 3, 4, 2, 128))
    wqk8 = np.ascontiguousarray(
        wqk[:, :, 2, NKT, :].transpose(1, 0, 2))  # [128, 2(qk), 128]

    # v projections pair-packed: pair p covers slots (2p, 2p+1); slot 4 is
    # alone in pair 2 (upper half zero).
    wv = np.zeros((128, 3, NKT, 128), np.float32)
    vbp = np.zeros((128, 3), np.float32)
    for slot in range(5):
        h = heads[slot]
        pair, sub = divmod(slot, 2)
        for p in range(NKT):
            wv[:, pair, p, 64 * sub:64 * sub + 64] = \
                W_qk_w[h * 192 + 128:h * 192 + 192,
                       p * 128:(p + 1) * 128].T
        vbp[64 * sub:64 * sub + 64, pair] = \
            W_qk_b[h * 192 + 128:h * 192 + 192]
    wv_f8 = np.ascontiguousarray(
        (wv * WSCALE).reshape(128, 3, 4, 2, 128))

    f8 = ml_dtypes.float8_e4m3
    return {
        "xt": xt.astype(f8),
        "xt8": xt8.astype(bf16),
        "qkdect": qkdecT.astype(bf16),
        "vdec": vdec.astype(bf16),
        "wqk": wqk_f8.astype(f8),
        "wqk8": wqk8.astype(bf16),
        "qkb": qkb,
        "wv": wv_f8.astype(f8),
        "vbp": vbp,
        "keep": keep_r,
        "lin": lin_in,
        "linb": lin_b16,
    }


def make_in_maps(dec_input, enc_input, mask, W_qk_w, W_qk_b, lin_w, lin_b):
    import ml_dtypes
    bf16 = ml_dtypes.bfloat16
    dec_input = np.asarray(dec_input, np.float32)
    enc_input = np.asarray(enc_input, np.float32)
    W_qk_w = np.asarray(W_qk_w, np.float32)
    W_qk_b = np.asarray(W_qk_b, np.float32)
    lin_w = np.asarray(lin_w, np.float32)
    lin_b = np.asarray(lin_b, np.float32)
    mask = np.asarray(mask)

    # keep mask with the global parity reorder applied to BOTH axes,
    # stored [128 partition, 16 ktile, S]
    keepT = (~mask).T.astype(np.float32)[np.ix_(_PERM, _PERM)]  # [k', q']
    keep_r = np.ascontiguousarray(
        keepT.reshape(16, 128, S).transpose(1, 0, 2)).astype(bf16)

    linT = np.ascontiguousarray(lin_w.T)  # [1024 (j,d), 1024 (n)]
    lin_in = np.ascontiguousarray(
        linT.reshape(8, 128, D).transpose(1, 0, 2)).astype(bf16)
    lin_b16 = lin_b.reshape(1, D).astype(bf16)

    in_maps = []
    for c in range(NCORES):
        b, hh = c // 2, c % 2
        in_maps.append(_prep_core_inputs(
            b, hh, dec_input, enc_input, keep_r, W_qk_w, W_qk_b,
            lin_in, lin_b16))
    return in_maps


def get_nc():
    if "nc" not in _CACHE:
        _CACHE["nc"] = _build_nc()
    return _CACHE["nc"]


def gather_output(results):
    out = np.empty((B, S, D), np.float32)
    for c in range(NCORES):
        b, hh = c // 2, c % 2
        heads = _heads_for(hh)
        co = results[c]["out"]  # [8, 128, 1024] bf16
        for slot, h in enumerate(heads):
            out[b, h * 128:(h + 1) * 128, :] = co[slot].astype(np.float32)
    return out


def kernel(dec_input, enc_input, mask, W_qk_w, W_qk_b, lin_w, lin_b):
    from concourse.bass_utils import run_bass_kernel_spmd

    nc = get_nc()
    in_maps = make_in_maps(dec_input, enc_input, mask, W_qk_w, W_qk_b,
                           lin_w, lin_b)
    res = run_bass_kernel_spmd(nc, in_maps, list(range(NCORES)))
    return gather_output(res.results)



# revision 17
# speedup vs baseline: 1.1258x; 1.1258x over previous
"""MultiHeadCrossAttention TRN2 kernel v2 (8 NeuronCores, SPMD).

Sharding: core c -> (batch b = c // 2, head-half hh = c % 2).
Head-half hh owns heads [hh, hh+2, ..., hh+14].

v2 redesign vs v1:
- q-major attention: att psum tiles are [128 q, 65] (K=128 contraction,
  N=65) -- half the PE cycles of the old [65, 2048] layout, and
  normalization becomes a per-partition tensor_scalar multiply.
- probs columns parity-reordered (col = 512 e + u for q = 1024 half + 2u + e)
  so each contiguous 128-wide probs slice is a single parity class; the
  post-att PE transpose (output partition base 64 e) lands directly in the
  output-linear contraction layout attn_t[(64 e + d), jj, r].
- keep mask resident in SBUF, reordered to match -- no per-head keep DMA.
- q processed in two halves; attention split into pass1 (kt 0-7) and pass2
  (kt 8-15) with one PSUM accumulation group per 2 KB zero-region.
- ACT-paced position loop: scores emitted one position ahead; all other PE
  work (att passes, projections, output linear) flows through a fine-grained
  work queue pumped between positions so neither PE nor ACT head-of-line
  blocks.
- engine split: PE matmuls only; ACT exp; DVE mask-mult + recip + normalize;
  Pool (gpsimd) bias adds + PSUM->SBUF copies.
"""

from collections import deque

import numpy as np

B, S, D, H, HD = 4, 2048, 1024, 16, 64
NCORES = 8
NKT = D // 128  # 8 K-tiles over the enc feature dim

_CACHE = {}
FG_BUDGET = 3
BG_BUDGET = 2
POOL_KT = 3


def _heads_for(hh):
    return list(range(hh, H, 2))


# global parity permutation: new index s' = e*1024 + u  <->  old s = 2u + e
_PERM = np.arange(S).reshape(1024, 2).T.reshape(S)  # old s for each new s'


def _build_nc(nslots=8, nreps=1, probs_bufs=13, kt_slice=True, debug=False,
              debug_slot=0):
    import concourse.bass as bass
    import concourse.tile as tile
    from concourse import bacc, mybir

    f32 = mybir.dt.float32
    bf16 = mybir.dt.bfloat16
    ts, ds = bass.ts, bass.ds
    Exp = mybir.ActivationFunctionType.Exp

    nc = bacc.Bacc("TRN2", target_bir_lowering=False, debug=False,
                   num_devices=NCORES)

    xt_d = nc.dram_tensor("xt", [128, 9, S], bf16, kind="ExternalInput")
    wqk_d = nc.dram_tensor("wqk", [2, 128, 3, NKT + 1, 128], bf16,
                           kind="ExternalInput")
    qkb_d = nc.dram_tensor("qkb", [2, 128, 3], f32, kind="ExternalInput")
    wv_d = nc.dram_tensor("wv", [128, 3, NKT, 128], bf16,
                          kind="ExternalInput")
    vbp_d = nc.dram_tensor("vbp", [128, 3], f32, kind="ExternalInput")
    qkdecT_d = nc.dram_tensor("qkdect", [2, 128, S], bf16,
                              kind="ExternalInput")
    vdec_d = nc.dram_tensor("vdec", [128, 16, 3, 65], bf16,
                            kind="ExternalInput")
    keep_d = nc.dram_tensor("keep", [128, 16, S], bf16, kind="ExternalInput")
    lin_d = nc.dram_tensor("lin", [128, 8, D], bf16, kind="ExternalInput")
    linb_d = nc.dram_tensor("linb", [1, D], bf16, kind="ExternalInput")
    out_d = nc.dram_tensor("out", [8, 128, D], bf16, kind="ExternalOutput")
    if debug:
        dqkt_d = nc.dram_tensor("dqkt", [128, S], bf16, kind="ExternalOutput")
        dprobs_d = nc.dram_tensor("dprobs", [16, 128, 1024], bf16,
                                  kind="ExternalOutput")
        dattn_d = nc.dram_tensor("dattn", [128, 8, 128], bf16,
                                 kind="ExternalOutput")
        dvaug_d = nc.dram_tensor("dvaug", [128, 16, 65], bf16,
                                 kind="ExternalOutput")
        dan_d = nc.dram_tensor("dan", [8, 128, 64], bf16,
                               kind="ExternalOutput")
        dpa_d = nc.dram_tensor("dpa", [8, 128, 65], f32,
                               kind="ExternalOutput")

    with tile.TileContext(nc) as tc:
        with (
            tc.tile_pool(name="consts", bufs=1) as consts,
            tc.tile_pool(name="qkt", bufs=4) as qktp,
            tc.tile_pool(name="vt2", bufs=1) as vt2p,  # [128, 1024] halves
            tc.tile_pool(name="vaug", bufs=3) as vaugp,
            tc.tile_pool(name="probs", bufs=probs_bufs) as probsp,
            tc.tile_pool(name="anorm", bufs=2) as anormp,
            tc.tile_pool(name="attn", bufs=2) as attnp,
            tc.tile_pool(name="rst", bufs=1) as rstp,
            tc.tile_pool(name="sums", bufs=2) as sumsp,
            tc.tile_pool(name="ps_sc", bufs=2, space="PSUM") as ps_sc,
            tc.tile_pool(name="ps_pa", bufs=1, space="PSUM") as ps_pa,
            tc.tile_pool(name="ps_pj", bufs=1, space="PSUM") as ps_pj,
            tc.tile_pool(name="ps_pg", bufs=1, space="PSUM") as ps_pg,
        ):
            # ---- resident constants.  DMA issue order follows first use:
            # slots 6,7 (dec-sourced) run first, so qkdecT/vdec/keep lead;
            # xt/wqk (needed by the first projections, pulled from slot 6
            # half 1 on) follow; lin/linb are only needed at slot ends. ----
            # dec-sourced heads: q-plane tile and k-plane tile, slot 6 on
            # partitions 0:64 and slot 7 on 64:128 of each
            qdec_sb = consts.tile([128, S], bf16)
            nc.sync.dma_start(out=qdec_sb[:], in_=qkdecT_d[0, :, :])
            kdec_sb = consts.tile([128, S], bf16)
            nc.sync.dma_start(out=kdec_sb[:], in_=qkdecT_d[1, :, :])
            vdec_sb = consts.tile([128, 16, 3, 65], bf16)
            nc.sync.dma_start(out=vdec_sb[:], in_=vdec_d[:, :, :, :])
            keep_sb = consts.tile([128, 16, S], bf16)
            for i in range(4):
                nc.sync.dma_start(out=keep_sb[:, 4 * i:4 * i + 4, :],
                                  in_=keep_d[:, 4 * i:4 * i + 4, :])
            xt_sb = consts.tile([128, 9, S], bf16)
            nc.sync.dma_start(out=xt_sb[:, 0:5, :], in_=xt_d[:, 0:5, :])
            nc.sync.dma_start(out=xt_sb[:, 5:9, :], in_=xt_d[:, 5:9, :])
            wqkq_sb = consts.tile([128, 3, NKT + 1, 128], bf16)
            nc.sync.dma_start(out=wqkq_sb[:], in_=wqk_d[0, :, :, :, :])
            wqkk_sb = consts.tile([128, 3, NKT + 1, 128], bf16)
            nc.sync.dma_start(out=wqkk_sb[:], in_=wqk_d[1, :, :, :, :])
            qkbq_sb = consts.tile([128, 3], f32)
            nc.sync.dma_start(out=qkbq_sb[:], in_=qkb_d[0, :, :])
            qkbk_sb = consts.tile([128, 3], f32)
            nc.sync.dma_start(out=qkbk_sb[:], in_=qkb_d[1, :, :])
            wv_sb = consts.tile([128, 3, NKT, 128], bf16)
            nc.sync.dma_start(out=wv_sb[:], in_=wv_d[:, :, :, :])
            vbp_sb = consts.tile([128, 3], f32)
            nc.sync.dma_start(out=vbp_sb[:], in_=vbp_d[:, :])
            lin_sb = consts.tile([128, 8, D], bf16)
            nc.sync.dma_start(out=lin_sb[:], in_=lin_d[:, :, :])
            linb_sb = consts.tile([1, D], bf16)
            nc.sync.dma_start(out=linb_sb[:], in_=linb_d[:, :])
            ones_sb = consts.tile([1, 128], bf16)
            nc.vector.memset(ones_sb[:], 1.0)
            ident_sb = consts.tile([128, 128], bf16)
            from concourse.masks import make_identity
            make_identity(nc, ident_sb[:])

            QK = {}    # slot % 2 -> (qkT, kT-or-None)
            VAUG = {}  # slot -> vaug tile
            GENS = {}  # ("qk", slot) / ("v", pair) -> generator in `work`
            debug_ref = ((debug_slot, 0), dan_d, dpa_d) if debug else None

            def xt_idx(slot, p):
                # slot 5 pass 7 reads the per-core extra tile (index 8)
                return p if not (slot == 5 and p == NKT - 1) else 8

            def gen_qkproj(pair):
                """paired qk projection for slots (2 pair, 2 pair + 1):
                qtile [128, S] = (qA on 0:64, qB on 64:128), ktile likewise.
                The extra contraction chunk NKT feeds xt[8] (slot-5 identity
                path on head-half 1; zero weights elsewhere)."""
                qtile = qktp.tile([128, S], bf16, tag="qkt")
                ktile = qktp.tile([128, S], bf16, tag="qkt")
                for qk in range(2):
                    dst = qtile if qk == 0 else ktile
                    wsb = wqkq_sb if qk == 0 else wqkk_sb
                    bsb = qkbq_sb if qk == 0 else qkbk_sb
                    for c in range(4):
                        pq = ps_pj.tile([128, 512], f32, tag="ps_pj")
                        for p in range(NKT + 1):
                            nc.tensor.matmul(
                                pq[:],
                                lhsT=wsb[:, pair, p, :],
                                rhs=xt_sb[:, min(p, 8), ds(c * 512, 512)],
                                start=(p == 0), stop=(p == NKT),
                                skip_group_check=True,
                            )
                            if p % 2 == 1:
                                yield
                        nc.vector.tensor_scalar_add(
                            dst[:, ds(c * 512, 512)], pq[:],
                            bsb[:, pair:pair + 1],
                        )
                QK[pair] = (qtile, ktile)

            def gen_vproj(pair):
                """pair-packed v projection -> vaug tiles for slots
                (2 pair, 2 pair + 1); fine-grained yields."""
                vaugA = vaugp.tile([128, 16, 65], bf16, tag="vaug")
                vaugB = vaugp.tile([128, 16, 65], bf16, tag="vaug")
                nc.gpsimd.memset(vaugA[:, :, 64:65], 1.0)
                nc.gpsimd.memset(vaugB[:, :, 64:65], 1.0)
                for ch in range(2):
                    vT2 = vt2p.tile([128, 1024], bf16, tag="vt2")
                    for cc in range(2):
                        c = 2 * ch + cc
                        pv = ps_pj.tile([128, 512], f32, tag="ps_pj")
                        for p in range(NKT):
                            nc.tensor.matmul(
                                pv[:],
                                lhsT=wv_sb[:, pair, p, :],
                                rhs=xt_sb[:, p, ds(c * 512, 512)],
                                start=(p == 0), stop=(p == NKT - 1),
                                skip_group_check=True,
                            )
                            if p % 2 == 1:
                                yield
                        nc.vector.tensor_scalar_add(
                            vT2[:, ds(cc * 512, 512)], pv[:],
                            vbp_sb[:, pair:pair + 1],
                        )
                    for tq in range(2 * ch, 2 * ch + 2):
                        pt = ps_pg.tile([128, 4, 128], bf16, tag="ps_pg")
                        for t in range(4):
                            nc.tensor.transpose(
                                pt[:, t, :],
                                vT2[:, ds((4 * tq + t) * 128 - ch * 1024,
                                          128)],
                                ident_sb[:, :])
                        nc.vector.tensor_copy(vaugA[:, ds(4 * tq, 4), 0:64],
                                              pt[:, :, 0:64])
                        nc.vector.tensor_copy(vaugB[:, ds(4 * tq, 4), 0:64],
                                              pt[:, :, 64:128])
                        yield
                VAUG[2 * pair] = vaugA
                VAUG[2 * pair + 1] = vaugB

            def vaug_for(slot):
                if slot < 5:
                    return VAUG[slot]
                return vdec_sb[:, :, slot - 5, :]

            def qk_aps(slot):
                if slot >= 6:
                    qtile, ktile = qdec_sb, kdec_sb
                    sub = slot - 6
                else:
                    qtile, ktile = QK[slot // 2]
                    sub = slot % 2
                lo = 64 * sub
                return qtile[lo:lo + 64, :], ktile[lo:lo + 64, :]

            def emit_scores(slot, half, kt, psc):
                qT, kT = qk_aps(slot)
                for cc in range(2):
                    nc.tensor.matmul(
                        psc[:, ds(cc * 512, 512)],
                        lhsT=kT[:, ts(kt, 128)],
                        rhs=qT[:, ds(half * 1024 + cc * 512, 512)],
                        start=True, stop=True, skip_group_check=True,
                    )

            def gen_att_pass1(slot, probs_tiles, pa_all, kts):
                # PSUM zero-regions are 2048 B: pa_all[:, 0:4, :] is one
                # region, pa_all[:, 4:8, :] the other.  start marks the WHOLE
                # region zero-on-first-write, so only the first group per
                # region (u = 0, 4) starts; only the last (u = 3, 7) stops.
                vaug = vaug_for(slot)
                for u in range(8):
                    pa = pa_all[:, u, :]
                    for kt in kts:
                        nc.tensor.matmul(
                            pa[:, 0:65],
                            lhsT=probs_tiles[kt][:, ts(u, 128)],
                            rhs=vaug[:, kt, :],
                            start=(kt == 0 and u % 4 == 0), stop=False,
                            skip_group_check=True,
                        )
                    yield

            def gen_att_pass2(slot, half, probs_tiles, pa_all, attn_t):
                # with the global parity reorder, e == half: partitions
                # 64*half of attn_t, r covered 0:128 across the 8 u-tiles
                vaug = vaug_for(slot)
                for tq in range(2):
                    pt = ps_pg.tile([128, 4, 128], bf16, tag="ps_pg")
                    for t in range(4):
                        u = 4 * tq + t
                        pa = pa_all[:, u, :]
                        for kt in range(12, 16):
                            nc.tensor.matmul(
                                pa[:, 0:65],
                                lhsT=probs_tiles[kt][:, ts(u, 128)],
                                rhs=vaug[:, kt, :],
                                start=False, stop=(kt == 15 and t == 3),
                                skip_group_check=True,
                            )
                        rc = sumsp.tile([128, 1], f32, tag="sums")
                        nc.vector.reciprocal(rc[:], pa[:, 64:65])
                        an = anormp.tile([128, 64], bf16, tag="anorm")
                        nc.vector.tensor_scalar_mul(an[:], pa[:, 0:64], rc[:])
                        nc.tensor.transpose(pt[ds(64 * half, 64), t, :],
                                            an[:], ident_sb[:, :])
                        if debug_ref and debug_ref[0] == (slot, half):
                            dstg = rstp.tile([128, 65], f32, tag="rst")
                            nc.vector.tensor_copy(dstg[:], pa[:, 0:65])
                            nc.sync.dma_start(out=debug_ref[2][u, :, :],
                                              in_=dstg[:])
                            nc.sync.dma_start(out=debug_ref[1][u, :, :],
                                              in_=an[:])
                        yield
                    # combined copy: src[p, t, (r j)] ->
                    # attn_t[64*half + p, j, 64*tq + 16 t + r]
                    csrc = pt[ds(64 * half, 64), :, :].rearrange(
                        "p t (r j) -> p j t r", j=8)
                    cdst = attn_t[ds(64 * half, 64), :,
                                  ds(64 * tq, 64)].rearrange(
                        "p j (t r) -> p j t r", r=16)
                    nc.vector.tensor_copy(cdst, csrc)
                    yield

            def gen_linear(slot, attn_t):
                if debug_ref and slot == debug_ref[0][0]:
                    nc.sync.dma_start(out=dattn_d[:], in_=attn_t[:])
                for n in range(2):
                    pr = ps_pg.tile([128, 512], f32, tag="ps_pg")
                    nc.tensor.matmul(
                        pr[:],
                        lhsT=ones_sb[0:1, :],
                        rhs=linb_sb[0:1, ds(n * 512, 512)],
                        start=True, stop=False, skip_group_check=True,
                    )
                    for jj in range(8):
                        nc.tensor.matmul(
                            pr[:],
                            lhsT=attn_t[:, jj, :],
                            rhs=lin_sb[:, jj, ds(n * 512, 512)],
                            start=False, stop=(jj == 7),
                            skip_group_check=True,
                        )
                        if jj % 3 == 2:
                            yield
                    rst = rstp.tile([128, 512], bf16, tag="rst")
                    nc.vector.tensor_copy(rst[:], pr[:])
                    nc.sync.dma_start(out=out_d[slot, :, ds(n * 512, 512)],
                                      in_=rst[:])
                    yield

            # ---------------- ACT-paced position scheduler -----------------
            fg = deque()
            bg = deque()

            def pump_q(q, n):
                done = 0
                while done < n and q:
                    if next(q[0], "DONE") == "DONE":
                        q.popleft()
                    else:
                        done += 1

            def force_drain(key):
                # Exhaust the producer generator for `key` before a consumer
                # binds to its outputs (QK/VAUG are late-set dicts; emitting a
                # reader before the producer finishes would silently bind to
                # a stale tile).
                g = GENS.pop(key, None)
                if g is None:
                    return
                while g in bg:
                    pump_q(bg, 1)
                while g in fg:
                    pump_q(fg, 1)

            def drain_gen(g):
                for _ in g:
                    pass

            ORDER = [6, 7, 0, 1, 2, 3, 4, 5][:nslots] if nslots == 8 \
                else list(range(nslots))
            VPOS = {7: 0, 3: 1, 5: 2}  # position -> v pair built there

            QKPOS = {1: 0, 2: 1, 4: 2}  # position -> qk pair built there

            def enqueue_bg(si, rep):
                qp = QKPOS.get(si)
                if qp is not None and 2 * qp < min(nslots, 6):
                    g = gen_qkproj(qp)
                    GENS["qk", qp] = g
                    bg.append(g)
                pair = VPOS.get(si)
                if pair is not None and 2 * pair < min(nslots, 5):
                    g = gen_vproj(pair)
                    GENS["v", pair] = g
                    bg.append(g)

            pending_psc = None
            for rep in range(nreps):
                if rep == 0:
                    pending_psc = ps_sc.tile([128, 1024], f32, tag="ps_sc")
                    emit_scores(ORDER[0], 0, 0, pending_psc)
                for si, slot in enumerate(ORDER):
                    enqueue_bg(si, rep)
                    if rep == 0 and si == 1:
                        # steady-state builds v pair 0 at si=7 of the prior
                        # rep; bootstrap it here for the first rep
                        g = gen_vproj(0)
                        GENS["v", 0] = g
                        bg.append(g)

                    attn_t = attnp.tile([128, 8, 128], bf16, tag="attn")
                    for half in range(2):
                        probs_tiles = []
                        pa_all = ps_pa.tile([128, 8, 128], f32, tag="ps_pa")
                        for kt in range(16):
                            psc = pending_psc
                            pk = probsp.tile([128, 1024], bf16, tag="probs")
                            probs_tiles.append(pk)
                            nc.scalar.activation(
                                out=pk[:], in_=psc[:], func=Exp, scale=0.125,
                            )
                            meng = nc.gpsimd if kt < POOL_KT else nc.vector
                            meng.tensor_mul(
                                pk[:], pk[:],
                                keep_sb[:, kt, ds(half * 1024, 1024)],
                            )
                            if debug and rep == 0 and slot == debug_slot \
                                    and half == 0:
                                nc.sync.dma_start(out=dprobs_d[kt, :, :],
                                                  in_=pk[:])
                            # software-pipelined next scores tile
                            if kt < 15:
                                nxt = (slot, half, kt + 1)
                            elif half == 0:
                                nxt = (slot, 1, 0)
                            elif si + 1 < len(ORDER):
                                nxt = (ORDER[si + 1], 0, 0)
                            elif rep + 1 < nreps:
                                nxt = (ORDER[0], 0, 0)
                            else:
                                nxt = None
                            if nxt is not None:
                                pending_psc = ps_sc.tile([128, 1024], f32,
                                                         tag="ps_sc")
                                emit_scores(*nxt, pending_psc)
                            if kt == 4 and slot < 5:
                                force_drain(("v", slot // 2))
                            if kt == 5:
                                fg.append(gen_att_pass1(
                                    slot, probs_tiles, pa_all, range(0, 4)))
                            elif kt == 9:
                                fg.append(gen_att_pass1(
                                    slot, probs_tiles, pa_all, range(4, 8)))
                            elif kt == 13:
                                fg.append(gen_att_pass1(
                                    slot, probs_tiles, pa_all, range(8, 12)))
                            elif kt == 10 and half == 1 and si + 1 < len(ORDER):
                                ns = ORDER[si + 1]
                                if ns < 6:
                                    force_drain(("qk", ns // 2))
                            elif kt == 15:
                                fg.append(gen_att_pass2(
                                    slot, half, probs_tiles, pa_all, attn_t))
                                if half == 1:
                                    fg.append(gen_linear(slot, attn_t))
                            pump_q(fg, FG_BUDGET)
                            pump_q(bg, BG_BUDGET)
                    if debug and rep == 0 and slot == debug_slot:
                        qkT0, kT0, _ = qk_aps(slot)
                        nc.sync.dma_start(out=dqkt_d[0:64, :], in_=qkT0)
                        nc.sync.dma_start(out=dqkt_d[64:128, :], in_=kT0)
                        nc.sync.dma_start(out=dvaug_d[:], in_=vaug_for(slot))
            while fg or bg:
                pump_q(fg, 1)
                pump_q(bg, 1)

    nc.compile()
    return nc


def _prep_core_inputs(b, hh, dec_input, enc_input, keep_r, W_qk_w, W_qk_b,
                      lin_in, lin_b16):
    import ml_dtypes
    bf16 = ml_dtypes.bfloat16
    heads = _heads_for(hh)
    enc_b = enc_input[b]
    dec_b = dec_input[b]
    encT = np.ascontiguousarray(enc_b.T[:, _PERM])  # [1024, 2048], s reordered

    xt = np.empty((9, 128, S), np.float32)
    xt[:NKT] = encT.reshape(NKT, 128, S)
    if hh == 0:
        xt[8] = encT[896:1024]
    else:
        # head 11 q,k dec cols 64:192 transposed
        xt[8] = np.ascontiguousarray(dec_b[:, 64:192].T[:, _PERM])
    xt = np.ascontiguousarray(xt.transpose(1, 0, 2))  # [128, 9, S]

    # [2(q/k), 128, S]: plane 0 = q (slot 6 on partitions 0:64, slot 7 on
    # 64:128), plane 1 = k likewise
    qkdecT = np.empty((2, 128, S), np.float32)
    for i, slot in enumerate((6, 7)):
        h = heads[slot]
        mc = h * 192 - 2 * D  # dec col offset of this head's q
        qkdecT[0, 64 * i:64 * i + 64] = dec_b[:, mc:mc + 64].T[:, _PERM]
        qkdecT[1, 64 * i:64 * i + 64] = dec_b[:, mc + 64:mc + 128].T[:, _PERM]

    vdec = np.empty((128, 16, 3, 65), np.float32)
    vdec[:, :, :, 64] = 1.0
    for blk, slot in enumerate((5, 6, 7)):
        h = heads[slot]
        mcv = h * 192 + 128 - 2 * D
        vcols = dec_b[_PERM, mcv:mcv + 64]  # [2048, 64], k reordered
        vdec[:, :, blk, 0:64] = vcols.reshape(16, 128, 64).transpose(1, 0, 2)

    # paired qk weights: wqk[qk, :, pair, chunk, 0:64] = slot 2*pair,
    # [.., 64:128] = slot 2*pair+1.  Chunk NKT is the extra xt[8] chunk
    # (identity for the hh=1 slot-5 path, zero otherwise).
    wqk = np.zeros((2, 128, 3, NKT + 1, 128), np.float32)
    qkb = np.zeros((2, 128, 3), np.float32)
    for slot in range(6):
        h = heads[slot]
        pair, sub = divmod(slot, 2)
        for qk in range(2):
            off = h * 192 + 64 * qk
            if hh == 1 and slot == 5:
                # q/k from dec cols (xt[8] holds dec[:, 64:192].T):
                # q = rows 0:64 of xt[8], k = rows 64:128
                wqk[qk, 64 * qk:64 * qk + 64, pair, NKT,
                    64 * sub:64 * sub + 64] = np.eye(64, dtype=np.float32)
            else:
                for p in range(NKT):
                    wqk[qk, :, pair, p, 64 * sub:64 * sub + 64] = \
                        W_qk_w[off:off + 64, p * 128:(p + 1) * 128].T
                qkb[qk, 64 * sub:64 * sub + 64, pair] = \
                    W_qk_b[off:off + 64]

    # v projections pair-packed: pair p covers slots (2p, 2p+1); slot 4 is
    # alone in pair 2 (upper half zero).
    wv = np.zeros((128, 3, NKT, 128), np.float32)
    vbp = np.zeros((128, 3), np.float32)
    for slot in range(5):
        h = heads[slot]
        pair, sub = divmod(slot, 2)
        for p in range(NKT):
            wv[:, pair, p, 64 * sub:64 * sub + 64] = \
                W_qk_w[h * 192 + 128:h * 192 + 192,
                       p * 128:(p + 1) * 128].T
        vbp[64 * sub:64 * sub + 64, pair] = \
            W_qk_b[h * 192 + 128:h * 192 + 192]

    return {
        "xt": xt.astype(bf16),
        "qkdect": qkdecT.astype(bf16),
        "vdec": vdec.astype(bf16),
        "wqk": wqk.astype(bf16),
        "qkb": qkb,
        "wv": wv.astype(bf16),
        "vbp": vbp,
        "keep": keep_r,
        "lin": lin_in,
        "linb": lin_b16,
    }


def make_in_maps(dec_input, enc_input, mask, W_qk_w, W_qk_b, lin_w, lin_b):
    import ml_dtypes
    bf16 = ml_dtypes.bfloat16
    dec_input = np.asarray(dec_input, np.float32)
    enc_input = np.asarray(enc_input, np.float32)
    W_qk_w = np.asarray(W_qk_w, np.float32)
    W_qk_b = np.asarray(W_qk_b, np.float32)
    lin_w = np.asarray(lin_w, np.float32)
    lin_b = np.asarray(lin_b, np.float32)
    mask = np.asarray(mask)

    # keep mask with the global parity reorder applied to BOTH axes,
    # stored [128 partition, 16 ktile, S]
    keepT = (~mask).T.astype(np.float32)[np.ix_(_PERM, _PERM)]  # [k', q']
    keep_r = np.ascontiguousarray(
        keepT.reshape(16, 128, S).transpose(1, 0, 2)).astype(bf16)

    linT = np.ascontiguousarray(lin_w.T)  # [1024 (j,d), 1024 (n)]
    lin_in = np.ascontiguousarray(
        linT.reshape(8, 128, D).transpose(1, 0, 2)).astype(bf16)
    lin_b16 = lin_b.reshape(1, D).astype(bf16)

    in_maps = []
    for c in range(NCORES):
        b, hh = c // 2, c % 2
        in_maps.append(_prep_core_inputs(
            b, hh, dec_input, enc_input, keep_r, W_qk_w, W_qk_b,
            lin_in, lin_b16))
    return in_maps


def get_nc():
    if "nc" not in _CACHE:
        _CACHE["nc"] = _build_nc()
    return _CACHE["nc"]


def gather_output(results):
    out = np.empty((B, S, D), np.float32)
    for c in range(NCORES):
        b, hh = c // 2, c % 2
        heads = _heads_for(hh)
        co = results[c]["out"]  # [8, 128, 1024] bf16
        for slot, h in enumerate(heads):
            out[b, h * 128:(h + 1) * 128, :] = co[slot].astype(np.float32)
    return out


def kernel(dec_input, enc_input, mask, W_qk_w, W_qk_b, lin_w, lin_b):
    from concourse.bass_utils import run_bass_kernel_spmd

    nc = get_nc()
    in_maps = make_in_maps(dec_input, enc_input, mask, W_qk_w, W_qk_b,
                           lin_w, lin_b)
    res = run_bass_kernel_spmd(nc, in_maps, list(range(NCORES)))
    return gather_output(res.results)

